# revision 29
# baseline (speedup 1.0000x reference)
"""Multi-head attention (B=4, S=2048, E=1024, H=16) on 8 TRN2 NeuronCores.

Sharding: batch x head-group tensor parallel -- core c = 2*b + hg handles
batch b and heads hg*8 .. hg*8+7 for ALL 2048 queries.  The output
projection is row-split; the host sums the per-pair partial outputs.

v3 design (vs the 317us ACT-bound baseline, see kernel_baseline.py):
  - Q/K/V projections in fp8 DoubleRow with 256-row contraction per
    instruction (4x fewer PE cycles than bf16).  x ships as fp8 hi+lo
    (residual scaled 2^4), weights as fp8 hi (+ 1/16-scaled lo
    companions for the x-lo pass; V adds a W-residual pass), all
    range-scaled into fp8's normal range; the inverse scales fold into
    the Q/K requantization multipliers and the exp scale.
  - exp split across TWO engines: most key-tile lines get exact exp on
    ACT; ~40% are computed on DVE via a Schraudolph bit trick (fp8e4
    bits of exp(y) ~= round(y*8*log2e + 55.6), a single fp32-PSUM ->
    uint8 tensor_scalar; HW-verified round-nearest + saturate-at-0 =
    exp underflow).  The softmax denominator sums the actual fp8
    p-values (ones column in V), so mixed pathways stay normalized
    (by-key plan, bias cancels with c=-0.4).
  - 512-query phases: 32 phases = 4 query groups x 8 heads, 16 key
    steps each.  The PV accumulator is then [128, 4, 65] = ONE psum
    bank, which frees banks for a FOUR-deep ring of [128, 512] score
    tiles: scores(s+4) WAR-waits exp(s) with ~4 steps of slack, so the
    exp->scores->exp latency chain (exp + ~770ns of sem hops and
    pipeline drains) no longer sets the step rate -- engines do.
  - projection/outproj psum double-buffered (2 banks), V evacuated in
    quads, transposes batched 4-wide into one bank + a single 2x-mode
    DVE copy per (d-tile, group).
"""

import sys

if "/opt/trn_rl_repo" not in sys.path:
    sys.path.insert(0, "/opt/trn_rl_repo")

import numpy as np
import ml_dtypes

B, S, E, H = 4, 2048, 1024, 16
P = 128
HD = 64           # head dim
NH = 8            # heads per core
DT = 4            # d-tiles (head pairs) per core
ET = E // P       # 8 e-tiles
EP = 4            # e-super-tiles of 256 (DoubleRow pairs)
ST = S // P       # 16 key tiles
NG = 4            # query groups of 512
N_CORES = 8
SCALE = 1.0 / float(np.sqrt(HD))
LOG2E = 1.4426950408889634
SCHRAUD_C = -0.4

_BF16 = ml_dtypes.bfloat16
_F8 = ml_dtypes.float8_e4m3

# number of DVE-computed exp lines per phase (of 16); by-key split with
# tuned c keeps the accuracy cost negligible
NDVE = [6, 6, 7, 6, 7, 6, 7, 7, 6, 7, 7, 6, 7, 7, 6, 7, 7, 6, 7, 7, 6, 7, 7, 6, 7, 7, 6, 7, 7, 6, 7, 6]


def _dve_js(n):
    return {int((i + 0.5) * 16 / n) for i in range(n)}


DVE_SET = {(p, j) for p in range(32) for j in _dve_js(NDVE[p])}

_cached = None


def _build(cfg):
    import concourse.bass as bass
    import concourse.tile as tile
    import concourse.mybir as mybir
    from concourse import bacc

    dt = mybir.dt
    nc = bacc.Bacc("TRN2", target_bir_lowering=False, debug=False)

    sig_q = cfg["sig_q"]          # PSUM->qt8 multiplier  sq/(sx*sw)
    sig_k = cfg["sig_k"]
    inv_sxw = cfg["inv_sxw"]      # V psum -> V units
    lam = cfg["lam"]              # ACT exp scale  SCALE/(2*sq*sk)
    aexp = 8.0 * LOG2E * lam      # DVE schraudolph multiplier
    bexp = 56.0 + SCHRAUD_C

    xh8_d = nc.dram_tensor("xh8", [P, EP, 2, S], dt.float8e4, kind="ExternalInput").ap()
    xl8_d = nc.dram_tensor("xl8", [P, EP, 2, S], dt.float8e4, kind="ExternalInput").ap()
    wq8_d = nc.dram_tensor("wq8", [P, DT, EP, 2, P], dt.float8e4, kind="ExternalInput").ap()
    wq8l_d = nc.dram_tensor("wq8l", [P, DT, EP, 2, P], dt.float8e4, kind="ExternalInput").ap()
    wk8_d = nc.dram_tensor("wk8", [P, DT, EP, 2, P], dt.float8e4, kind="ExternalInput").ap()
    wk8l_d = nc.dram_tensor("wk8l", [P, DT, EP, 2, P], dt.float8e4, kind="ExternalInput").ap()
    wv8_d = nc.dram_tensor("wv8", [P, NH, EP, 2, HD], dt.float8e4, kind="ExternalInput").ap()
    wv8m_d = nc.dram_tensor("wv8m", [P, NH, EP, 2, HD], dt.float8e4, kind="ExternalInput").ap()
    wv8l_d = nc.dram_tensor("wv8l", [P, NH, EP, 2, HD], dt.float8e4, kind="ExternalInput").ap()
    wo_d = nc.dram_tensor("wo", [512, E], dt.bfloat16, kind="ExternalInput").ap()
    bq_d = nc.dram_tensor("bq", [P, DT], dt.float32, kind="ExternalInput").ap()
    bk_d = nc.dram_tensor("bk", [P, DT], dt.float32, kind="ExternalInput").ap()
    bv4_d = nc.dram_tensor("bv4", [1, NH * 256], dt.bfloat16, kind="ExternalInput").ap()
    bo_d = nc.dram_tensor("bo", [P, ET], dt.float32, kind="ExternalInput").ap()
    iden_d = nc.dram_tensor("iden", [P, P], dt.bfloat16, kind="ExternalInput").ap()
    out_d = nc.dram_tensor("out", [E, S], dt.bfloat16, kind="ExternalOutput").ap()

    DR = mybir.MatmulPerfMode.DoubleRow

    with tile.TileContext(nc) as tc:
        with (
            tc.tile_pool(name="const", bufs=1) as cpool,
            tc.tile_pool(name="acts", bufs=1) as apool,
            tc.tile_pool(name="pp", bufs=3) as ppool,
            tc.tile_pool(name="oqp", bufs=2) as oqpool,
            tc.tile_pool(name="recp", bufs=2) as recpool,
            tc.tile_pool(name="outs", bufs=8) as outpool,
            tc.tile_pool(name="pssc", bufs=4, space="PSUM") as scpool,   # 4 banks
            tc.tile_pool(name="pspv", bufs=2, space="PSUM") as pvpool,   # 2 banks
            tc.tile_pool(name="pspj", bufs=2, space="PSUM") as pjpool,   # 2 banks
        ):
            # ---------------- constants / inputs -----------------------
            xh8 = cpool.tile([P, EP, 2, S], dt.float8e4)
            xl8 = cpool.tile([P, EP, 2, S], dt.float8e4)
            wq8 = cpool.tile([P, DT, EP, 2, P], dt.float8e4)
            wq8l = cpool.tile([P, DT, EP, 2, P], dt.float8e4)
            wk8 = cpool.tile([P, DT, EP, 2, P], dt.float8e4)
            wk8l = cpool.tile([P, DT, EP, 2, P], dt.float8e4)
            wv8 = cpool.tile([P, NH, EP, 2, HD], dt.float8e4)
            wv8m = cpool.tile([P, NH, EP, 2, HD], dt.float8e4)
            wv8l = cpool.tile([P, NH, EP, 2, HD], dt.float8e4)
            wo = cpool.tile([P, DT, E], dt.bfloat16)
            bq = cpool.tile([P, DT], dt.float32)
            bk = cpool.tile([P, DT], dt.float32)
            bv4 = cpool.tile([1, NH * 256], dt.bfloat16)
            bo = cpool.tile([P, ET], dt.float32)
            iden = cpool.tile([P, P], dt.bfloat16)
            ones1 = cpool.tile([1, P], dt.bfloat16)

            qt8 = [apool.tile([P, S], dt.float8e4, name=f"qt8{t}") for t in range(DT)]
            kt8 = [apool.tile([P, S], dt.float8e4, name=f"kt8{t}") for t in range(DT)]
            va8h = [apool.tile([P, ST, 72], dt.float8e4, name=f"vh{h}") for h in range(NH)]
            va8l = [apool.tile([P, ST, 72], dt.float8e4, name=f"vl{h}") for h in range(NH)]
            scb = [apool.tile([P, S], dt.bfloat16, name=f"scb{t}") for t in range(DT)]

            rings = [
                scpool.tile([P, 512], dt.float32, tag="sc", name=f"ring{i}")
                for i in range(4)
            ]
            pv_of = {}

            def pv_tile(phase):
                if phase not in pv_of:
                    pv_of[phase] = pvpool.tile(
                        [P, 4, HD + 1], dt.float32, tag="pv", name=f"pv{phase}"
                    )
                return pv_of[phase]

            # DMA order = criticality.
            def wv_head(h):
                for wd, wt in ((wv8_d, wv8), (wv8m_d, wv8m), (wv8l_d, wv8l)):
                    nc.sync.dma_start(wt[:, h], wd[:, h])

            for wd, wt in ((wk8_d, wk8), (wk8l_d, wk8l), (wq8_d, wq8), (wq8l_d, wq8l)):
                nc.sync.dma_start(wt[:, 0], wd[:, 0])
            for ep in range(EP):
                nc.sync.dma_start(xh8[:, ep, :, 0:1024], xh8_d[:, ep, :, 0:1024])
            nc.sync.dma_start(bk[:], bk_d)
            nc.sync.dma_start(bq[:], bq_d)
            nc.sync.dma_start(xh8[:, :, :, 1024:1536], xh8_d[:, :, :, 1024:1536])
            for ep in range(EP):
                nc.sync.dma_start(xl8[:, ep, :, 0:1024], xl8_d[:, ep, :, 0:1024])
            wv_head(0)
            nc.sync.dma_start(bv4[:], bv4_d)

            def wkq_tile(t):
                for wd, wt in (
                    (wk8_d, wk8), (wk8l_d, wk8l), (wq8_d, wq8), (wq8l_d, wq8l)
                ):
                    nc.sync.dma_start(wt[:, t], wd[:, t])

            nc.sync.dma_start(xl8[:, :, :, 1024:1536], xl8_d[:, :, :, 1024:1536])
            nc.sync.dma_start(xh8[:, :, :, 1536:2048], xh8_d[:, :, :, 1536:2048])
            nc.sync.dma_start(xl8[:, :, :, 1536:2048], xl8_d[:, :, :, 1536:2048])
            wkq_tile(1)
            wv_head(1)
            wkq_tile(2)
            wv_head(2)
            wv_head(3)
            wkq_tile(3)
            for h in range(4, NH):
                wv_head(h)
            nc.sync.dma_start(iden[:], iden_d)
            nc.sync.dma_start(wo[:], wo_d.rearrange("(eo p) c -> p eo c", p=P))
            nc.sync.dma_start(bo[:], bo_d)
            nc.gpsimd.memset(ones1[:], 1.0)
            actwarm = cpool.tile([1, P], dt.float32)
            nc.scalar.activation(
                actwarm[:], ones1[:], mybir.ActivationFunctionType.Exp, scale=1.0
            )
            for h in range(NH):
                nc.gpsimd.memset(va8h[h][:, :, HD : HD + 1], 1.0)
                nc.gpsimd.memset(va8l[h][:, :, HD : HD + 1], 0.0)

            # ---------------- small-chunk emitters ----------------------
            def kq_halves(which, t, c):
                """fp8 DR projection chunk: part0 = hi pass, part1 = lo
                pass + fused scale+bias evac to fp8 (DVE)."""
                wh, wl = (wk8, wk8l) if which == "k" else (wq8, wq8l)
                b_, dst = (bk, kt8) if which == "k" else (bq, qt8)
                sg = sig_k if which == "k" else sig_q
                holder = {}

                def part0():
                    ps = pjpool.tile(
                        [P, 512], dt.float32, tag="pj", name=f"{which}{t}{c}"
                    )
                    holder[0] = ps
                    for ep in range(EP):
                        nc.tensor.matmul(
                            ps[:],
                            wh[:, t, ep, :, :],
                            xh8[:, ep, :, c * 512 : (c + 1) * 512],
                            start=(ep == 0),
                            stop=False,
                            perf_mode=DR,
                        )

                def part1():
                    ps = holder[0]
                    for ep in range(EP):
                        nc.tensor.matmul(
                            ps[:],
                            wl[:, t, ep, :, :],
                            xl8[:, ep, :, c * 512 : (c + 1) * 512],
                            start=False,
                            stop=(ep == EP - 1),
                            perf_mode=DR,
                        )
                    nc.vector.tensor_scalar(
                        dst[t][:, c * 512 : (c + 1) * 512], ps[:],
                        sg, b_[:, t : t + 1],
                        op0=mybir.AluOpType.mult, op1=mybir.AluOpType.add,
                    )

                return [part0, part1]

            def v_quad(h, q4):
                """V rows for key tiles 4*q4..4*q4+3 of head h."""
                holder = {}

                def sts(ps, sts_):
                    # start=True zero-fills the whole 2KB bank: only the
                    # quad's first matmul uses it
                    for st in sts_:
                        s = 4 * q4 + st
                        for pi, (x8, w8) in enumerate(
                            ((xh8, wv8), (xh8, wv8m), (xl8, wv8l))
                        ):
                            for ep in range(EP):
                                nc.tensor.matmul(
                                    ps[:, st * HD : (st + 1) * HD],
                                    x8[:, ep, :, s * P : (s + 1) * P],
                                    w8[:, h, ep, :, :],
                                    start=(st == 0 and pi == 0 and ep == 0),
                                    stop=False,
                                    perf_mode=DR,
                                    skip_group_check=True,
                                )

                def part0():
                    ps = pjpool.tile([P, 512], dt.float32, tag="pj", name=f"v{h}{q4}")
                    holder[0] = ps
                    sts(ps, (0, 1))

                def part1():
                    ps = holder[0]
                    sts(ps, (2, 3))
                    nc.tensor.matmul(
                        ps[:, 0:256],
                        ones1[0:1, :],
                        bv4[0:1, h * 256 : (h + 1) * 256],
                        start=False,
                        stop=True,
                        skip_group_check=True,
                    )
                    vtmp = recpool.tile([P, 4, HD], dt.float32, tag="vt", name=f"vt{h}{q4}")
                    nc.vector.tensor_scalar_mul(
                        vtmp[:].rearrange("p a b -> p (a b)"), ps[:, 0:256], inv_sxw
                    )
                    sl = slice(4 * q4, 4 * q4 + 4)
                    nc.gpsimd.tensor_copy(va8h[h][:, sl, 0:HD], vtmp[:])
                    nc.gpsimd.tensor_tensor(
                        va8l[h][:, sl, 0:HD], vtmp[:], va8h[h][:, sl, 0:HD],
                        mybir.AluOpType.subtract,
                    )

                return [part0, part1]

            def outproj_halves(eo, g):
                holder = {}
                q0 = g * 512

                def part0():
                    ps = pjpool.tile([P, 512], dt.float32, tag="pj", name=f"o{eo}{g}")
                    holder[0] = ps
                    for t in (0, 1):
                        nc.tensor.matmul(
                            ps[:],
                            wo[:, t, eo * P : (eo + 1) * P],
                            scb[t][:, q0 : q0 + 512],
                            start=(t == 0),
                            stop=False,
                        )

                def part1():
                    ps = holder[0]
                    for t in (2, 3):
                        nc.tensor.matmul(
                            ps[:],
                            wo[:, t, eo * P : (eo + 1) * P],
                            scb[t][:, q0 : q0 + 512],
                            start=False,
                            stop=(t == DT - 1),
                        )
                    ot = outpool.tile([P, 512], dt.bfloat16, tag="ot", name=f"oe{eo}{g}")
                    nc.vector.tensor_scalar_add(ot[:], ps[:], bo[:, eo : eo + 1])
                    nc.sync.dma_start(
                        out_d[eo * P : (eo + 1) * P, q0 : q0 + 512], ot[:]
                    )

                return [part0, part1]

            oq_tiles = {}

            def transpose_quad(t, g):
                """4 PE transposes into one bank + one 2x-mode DVE copy."""
                def go():
                    tp = pjpool.tile([P, 512], dt.float32, tag="pj", name=f"tp{t}{g}")
                    tpb = tp[:, 0:256].bitcast(dt.bfloat16)
                    for qt in range(4):
                        nc.tensor.transpose(
                            tpb[:, qt * P : (qt + 1) * P], oq_tiles[(t, g)][:, qt, :],
                            iden[:],
                        )
                    nc.vector.tensor_copy(
                        scb[t][:, g * 512 : (g + 1) * 512], tpb[:]
                    )
                return go

            # ---------------- attention stream ---------------------------
            def phase_of(step):
                phase, j = divmod(step, ST)
                g, h = divmod(phase, NH)
                return h, g, j

            def scores(step):
                h, g, j = phase_of(step)
                t, hp = h // 2, (h % 2) * HD
                rg = rings[step % 4]
                nc.tensor.matmul(
                    rg[:],
                    kt8[t][hp : hp + HD, j * P : (j + 1) * P]
                    .unsqueeze(1)
                    .broadcast_to((HD, 2, P)),
                    qt8[t][hp : hp + HD, g * 512 : (g + 1) * 512]
                    .unsqueeze(1)
                    .broadcast_to((HD, 2, 512)),
                    start=True,
                    stop=True,
                    perf_mode=DR,
                )

            def emit_evac(h, g):
                pv = pv_of[(g * NH + h)]
                t, half = h // 2, h % 2
                if half == 0:
                    oq_tiles[(t, g)] = oqpool.tile(
                        [P, 4, P], dt.bfloat16, tag="oq", name=f"oq{t}{g}"
                    )
                oq = oq_tiles[(t, g)]
                pvc = recpool.tile([P, 4, HD + 1], dt.float32, tag="pvc", name=f"pc{h}{g}")
                nc.scalar.activation(
                    pvc[:], pv[:, :, 0 : HD + 1],
                    mybir.ActivationFunctionType.Copy, scale=1.0,
                )
                rec = recpool.tile([P, 4], dt.float32, tag="rec", name=f"rc{h}{g}")
                scr = recpool.tile([P, 4], dt.float32, tag="scr", name=f"sr{h}{g}")
                nc.vector.reciprocal_approx_accurate(
                    rec[:], pvc[:, :, HD : HD + 1].rearrange("p a b -> p (a b)"), scr[:]
                )
                for qt in range(4):
                    nc.gpsimd.tensor_scalar(
                        oq[:, qt, half * HD : (half + 1) * HD],
                        pvc[:, qt, 0:HD],
                        rec[:, qt : qt + 1],
                        None,
                        op0=mybir.AluOpType.mult,
                    )

            def exp_emit(step, ppair):
                phase = step // ST
                j = step % ST
                dst = ppair[:, step % 2, :]
                src = rings[step % 4][:]
                if (phase, j) in DVE_SET:
                    nc.vector.tensor_scalar(
                        dst.bitcast(dt.uint8), src, aexp, bexp,
                        op0=mybir.AluOpType.mult, op1=mybir.AluOpType.add,
                    )
                else:
                    nc.scalar.activation(
                        dst, src, mybir.ActivationFunctionType.Exp, scale=lam
                    )

            def pv_wave(step, ppair_of):
                h, g, j = phase_of(step)
                pv = pv_tile(step // ST)
                ppair = ppair_of[step - 1]
                for qt in range(4):
                    lhsT = ppair[:, :, qt * P : (qt + 1) * P]
                    for vi, va8 in enumerate((va8h, va8l)):
                        nc.tensor.matmul(
                            pv[:, qt, 0 : HD + 1],
                            lhsT,
                            va8[h][:, j - 1 : j + 1, 0 : HD + 1],
                            start=(j == 1 and qt == 0 and vi == 0),
                            stop=(j == ST - 1 and vi == 1),
                            perf_mode=DR,
                            skip_group_check=True,
                        )

            def run_stream(work):
                STEP_BUDGET = 350.0
                n_steps = 16 * NH * NG
                scores(0)
                scores(1)
                pending = sorted(work, key=lambda w: (w["due"], w["release"]))
                current = None
                ppair = None
                ppair_of = {}
                pend_evac = None
                for step in range(n_steps):
                    h, g, j = phase_of(step)
                    if step % 2 == 0:
                        ppair = ppool.tile(
                            [P, 2, 512], dt.float8e4, tag="p", name=f"p{step}"
                        )
                        ppair_of[step] = ppair
                    exp_emit(step, ppair)
                    if pend_evac is not None:
                        emit_evac(*pend_evac)
                        pend_evac = None
                    if step + 2 < n_steps:
                        scores(step + 2)
                    # phase 0 defers the first two PV waves so the head-0 V
                    # quads (gated on the x fp8 DMA stream) have time
                    if step == 3:
                        pv_wave(1, ppair_of)
                        pv_wave(3, ppair_of)
                    elif step % 2 == 1 and step > 3:
                        pv_wave(step, ppair_of)
                    budget = STEP_BUDGET
                    while budget > 0:
                        if current is None:
                            cand = [w for w in pending if w["release"] <= step]
                            if not cand:
                                break
                            current = cand[0]
                            pending.remove(current)
                            assert current["due"] >= step, (
                                f"work item overdue: emitted step {step}, "
                                f"due {current['due']}"
                            )
                        fn, cost = current["fns"].pop(0)
                        fn()
                        budget -= cost
                        if not current["fns"]:
                            current = None
                    if j == ST - 1:
                        pend_evac = (h, g)
                if pend_evac is not None:
                    emit_evac(*pend_evac)
                    pend_evac = None
                leftovers = ([current] if current else []) + pending
                leftovers.sort(key=lambda w: (w["release"], w["due"]))
                for w in leftovers:
                    for fn, _ in w["fns"]:
                        fn()

            # ---------------- emission schedule -------------------------
            # Prefix: K d0 cols 0:1024 + Q d0 group 0 accumulate in ring
            # slots 0-2 (hi pass over ep, then lo), evacs ACT/DVE.
            pref = [
                ("k", 0, rings[0]), ("q", 0, rings[1]), ("k", 1, rings[2]),
            ]

            def pref_mm(pas, ep, which, c, rg):
                # hi pass only: the prefix K/Q chunks skip the x-lo term,
                # trading ~0.05e-2 of rel err for ~2.5us of prologue
                w8 = wk8 if which == "k" else wq8
                x8 = xh8
                nc.tensor.matmul(
                    rg[:],
                    w8[:, 0, ep, :, :],
                    x8[:, ep, :, c * 512 : (c + 1) * 512],
                    start=(ep == 0),
                    stop=(ep == EP - 1),
                    perf_mode=DR,
                )

            def pref_evac(which, c, rg, eng):
                b_, dst = (bk, kt8) if which == "k" else (bq, qt8)
                sg = sig_k if which == "k" else sig_q
                d_ap = dst[0][:, c * 512 : (c + 1) * 512]
                if eng == "act":
                    nc.scalar.activation(
                        d_ap, rg[:], mybir.ActivationFunctionType.Identity,
                        bias=b_[:, 0:1], scale=sg,
                    )
                else:
                    nc.vector.tensor_scalar(
                        d_ap, rg[:], sg, b_[:, 0:1],
                        op0=mybir.AluOpType.mult, op1=mybir.AluOpType.add,
                    )

            for ep in range(EP):
                for which, c, rg in pref:
                    pref_mm(0, ep, which, c, rg)
            pref_evac(*pref[0], "act")
            pref_evac(*pref[1], "dve")
            pref_evac(*pref[2], "act")

            KQC, VC, TRC, OPC = 430.0, 400.0, 300.0, 430.0
            work = []

            def add(release, due, fns, cost):
                work.append(
                    {"release": release, "due": due,
                     "fns": [(f, cost) for f in fns]}
                )

            # due = first-use wave step minus 1: the wave is emitted BEFORE
            # the thunk phase of its step, so a quad emitted at its due step
            # must still precede the wave in program order
            vrel0 = [0, 2, 6, 10]
            vdue0 = [2, 4, 8, 12]
            for q4 in range(4):
                add(vrel0[q4], vdue0[q4], v_quad(0, q4), VC)
            for h in range(1, NH):
                for q4 in range(4):
                    add(0, 16 * h + 4 * q4 - 1, v_quad(h, q4), VC)
            # (first use of quad q4 is the wave at step 16h+4q4+1, emitted
            # before thunks of that step: due 16h+4q4-1 emits in time)
            # K: d-tile 0 chunks 2/3 (0/1 in the prefix), d-tiles 1..3 all
            add(2, 5, kq_halves("k", 0, 2), KQC)
            add(5, 9, kq_halves("k", 0, 3), KQC)
            for t2 in range(1, DT):
                for c in range(4):
                    add(0, 32 * t2 + 4 * c - 3, kq_halves("k", t2, c), KQC)
            # Q(t, g): due before phase 8g+2t (t0 g0 in the prefix)
            for t2 in range(1, DT):
                add(0, 32 * t2 - 3, kq_halves("q", t2, 0), KQC)
            for g in range(1, NG):
                for t2 in range(DT):
                    add(0, 128 * g + 32 * t2 - 3, kq_halves("q", t2, g), KQC)
            # transpose quads: (t, g) ready after phase 8g+2t+1's evac
            for g in range(NG):
                for t2 in range(DT):
                    rel = 16 * (8 * g + 2 * t2 + 1) + 17
                    if g == NG - 1 and t2 == DT - 1:
                        add(10**6 - 1, 10**6, [transpose_quad(t2, g)], TRC)
                    else:
                        add(rel, rel + 15, [transpose_quad(t2, g)], TRC)
            # outproj group g after all its transposes (tail for g3)
            for g in range(NG - 1):
                for eo in range(ET):
                    rel = 16 * (8 * g + 8) + 20 + 5 * eo
                    add(rel, rel + 60, outproj_halves(eo, g), OPC)
            ci = [0]

            def tail_outproj(eo, g):
                def go():
                    ps = rings[ci[0] % 4][:]
                    ci[0] += 1
                    q0 = g * 512
                    for t in range(DT):
                        nc.tensor.matmul(
                            ps,
                            wo[:, t, eo * P : (eo + 1) * P],
                            scb[t][:, q0 : q0 + 512],
                            start=(t == 0),
                            stop=(t == DT - 1),
                        )
                    ot = outpool.tile([P, 512], dt.bfloat16, tag="ot", name=f"ot{eo}{g}")
                    if eo % 2 == 0:
                        nc.scalar.activation(
                            ot[:], ps, mybir.ActivationFunctionType.Identity,
                            bias=bo[:, eo : eo + 1], scale=1.0,
                        )
                    else:
                        nc.vector.tensor_scalar_add(ot[:], ps, bo[:, eo : eo + 1])
                    nc.sync.dma_start(
                        out_d[eo * P : (eo + 1) * P, q0 : q0 + 512], ot[:]
                    )
                return go

            for eo in range(ET):
                add(10**6, 10**6, [tail_outproj(eo, NG - 1)], OPC)

            run_stream(work)

    nc.compile()
    return nc


def _dr_layout(a2d):
    """[E, N] -> [P, EP, 2, N] DoubleRow pairing: row = ep*256 + s*128 + r."""
    E_, N_ = a2d.shape
    return np.ascontiguousarray(
        a2d.reshape(EP, 2, P, N_).transpose(2, 0, 1, 3)
    )


def _undr(a):
    """inverse of _dr_layout (fp32)."""
    P_, EP_, two, N_ = a.shape
    return a.astype(np.float32).transpose(1, 2, 0, 3).reshape(EP_ * two * P_, N_)


def _f8(a):
    return np.ascontiguousarray(a).astype(_F8)


def _prep(x, W_qkv, b_qkv, W_out, b_out):
    """Host-side sharding + fp8 layout prep. Returns (cfg, per-core maps)."""
    w = W_qkv.reshape(E, H, 3, HD)
    b3 = b_qkv.reshape(H, 3, HD)
    iden = np.eye(P, dtype=np.float32).astype(_BF16)

    sx = 224.0 / float(np.abs(x).max())
    sw = 224.0 / float(np.abs(W_qkv).max())
    wq_all = w[:, :, 0, :].reshape(E, E)
    wk_all = w[:, :, 1, :].reshape(E, E)
    bq_all = b3[:, 0, :].ravel()
    bk_all = b3[:, 1, :].ravel()
    Xf = x.reshape(-1, E).astype(np.float32)
    q_max = float(np.abs(Xf @ wq_all + bq_all).max())
    k_max = float(np.abs(Xf @ wk_all + bk_all).max())
    sq = 224.0 / q_max
    sk = 224.0 / k_max
    cfg = {
        "sig_q": sq / (sx * sw),
        "sig_k": sk / (sx * sw),
        "inv_sxw": 1.0 / (sx * sw),
        "lam": SCALE / (2.0 * sq * sk),
    }

    in_maps = []
    for core in range(N_CORES):
        b, hg = core // 2, core % 2
        hs = slice(hg * NH, (hg + 1) * NH)
        xt = np.ascontiguousarray(x[b].T).astype(np.float32)      # [E, S]
        xs = xt * sx
        xh8 = _f8(_dr_layout(xs))
        xl8 = _f8(_dr_layout((xs - _undr(xh8)) * 16.0))
        wq = np.ascontiguousarray(w[:, hs, 0, :].reshape(E, 512)).astype(np.float32) * sw
        wk = np.ascontiguousarray(w[:, hs, 1, :].reshape(E, 512)).astype(np.float32) * sw
        wv = np.ascontiguousarray(w[:, hs, 2, :].reshape(E, 512)).astype(np.float32) * sw
        def tmaj(a):      # [P, EP, 2, 512] -> [P, DT, EP, 2, 128]
            return np.ascontiguousarray(
                a.reshape(P, EP, 2, DT, P).transpose(0, 3, 1, 2, 4))

        def hmaj(a):      # [P, EP, 2, 512] -> [P, NH, EP, 2, 64]
            return np.ascontiguousarray(
                a.reshape(P, EP, 2, NH, HD).transpose(0, 3, 1, 2, 4))

        wq8 = _f8(_dr_layout(wq))
        wq8l = tmaj(_f8(_dr_layout(wq / 16.0)))
        wk8 = _f8(_dr_layout(wk))
        wk8l = tmaj(_f8(_dr_layout(wk / 16.0)))
        wv8 = _f8(_dr_layout(wv))
        wv8m = hmaj(_f8(_dr_layout(wv - _undr(wv8))))
        wv8lo = hmaj(_f8(_dr_layout(wv / 16.0)))
        wq8 = tmaj(wq8)
        wk8 = tmaj(wk8)
        wv8 = hmaj(wv8)
        wo = np.ascontiguousarray(W_out[hg * 512 : (hg + 1) * 512, :]).astype(_BF16)
        bqs = (np.ascontiguousarray(b3[hs, 0, :].reshape(DT, P).T) * sq).astype(np.float32)
        bks = (np.ascontiguousarray(b3[hs, 1, :].reshape(DT, P).T) * sk).astype(np.float32)
        bv = b3[hs, 2, :].reshape(NH, HD) * (sx * sw)
        bv4 = np.ascontiguousarray(
            np.tile(bv[:, None, :], (1, 4, 1)).reshape(1, NH * 256)
        ).astype(_BF16)
        bos = (np.ascontiguousarray(b_out.reshape(ET, P).T)
               * (1.0 if hg == 0 else 0.0)).astype(np.float32)
        in_maps.append(
            {
                "xh8": xh8, "xl8": xl8,
                "wq8": wq8, "wq8l": wq8l, "wk8": wk8, "wk8l": wk8l,
                "wv8": wv8, "wv8m": wv8m, "wv8l": wv8lo,
                "wo": wo, "bq": bqs, "bk": bks, "bv4": bv4, "bo": bos,
                "iden": iden,
            }
        )
    return cfg, in_maps


def run_raw(x, W_qkv, b_qkv, W_out, b_out, trace=False, **kw):
    global _cached
    from concourse.bass_utils import run_bass_kernel_spmd

    cfg, in_maps = _prep(
        np.asarray(x), np.asarray(W_qkv), np.asarray(b_qkv),
        np.asarray(W_out), np.asarray(b_out),
    )
    if _cached is None:
        _cached = _build(cfg)
    nc = _cached
    res = run_bass_kernel_spmd(
        nc, in_maps, core_ids=list(range(N_CORES)), trace=trace, **kw
    )
    out = np.empty((B, S, E), dtype=np.float32)
    for b in range(B):
        acc = np.asarray(res.results[2 * b]["out"]).astype(np.float32) + np.asarray(
            res.results[2 * b + 1]["out"]
        ).astype(np.float32)
        out[b] = acc.T
    return out, res


def kernel(x, W_qkv, b_qkv, W_out, b_out):
    out, _ = run_raw(x, W_qkv, b_qkv, W_out, b_out, trace=False)
    return out
